# revision 46
# baseline (speedup 1.0000x reference)
"""2-layer GCN (symmetric norm, self-loops, mean-pool, FC) on 8 TRN2 NeuronCores.

Strategy (dst-partitioned message passing, all compute in bf16/fp32-accum):
  - Nodes are partitioned by destination across the 8 cores (6250 real nodes
    per core, padded to 6272 = 49 blocks of 128). Each core owns the incident
    edges of its destination nodes (plus self-loops as ordinary edges with
    weight 1/deg).
  - Per layer the aggregation out[d] = sum_e w_e * h[src_e] is computed as a
    sequence of 128-edge chunks: the source rows are fetched with the gpsimd
    dma_gather primitive (int16 indices -> lo/hi table split at 32768), a
    one-hot-times-weight selection matrix S[e, d] is built on the vector
    engine from iota==dst, and the tensor engine accumulates
    aggT[f, d] += msg[e, f]^T @ S[e, d] in PSUM.
  - Layer 1 aggregates x first (128 wide), then applies W1/b1/ReLU and W2 so
    that both layers' message passing runs at 128 features. The per-node
    t2 = relu(agg1@W1+b1)@W2 shards are AllGathered between layers.
  - Mean-pool partials ([128] per core) are returned to the host, which
    applies the (tiny) exact pad correction, the mean, and the final FC.

The bass program is schedule-static: it depends on the runtime graph only
through the per-block-slot chunk budgets (max over cores), which the host
derives from edge_index before building/compiling.
"""

import os
import sys

import numpy as np

for _p in ("/opt/trn_rl_repo", "/root/.axon_site/_ro/trn_rl_repo"):
    if os.path.isdir(_p) and _p not in sys.path:
        sys.path.append(_p)

import ml_dtypes  # noqa: E402
import concourse.bacc as bacc  # noqa: E402
import concourse.mybir as mybir  # noqa: E402
from concourse.bass_utils import run_bass_kernel_spmd  # noqa: E402
from concourse.tile import TileContext  # noqa: E402
from contextlib import ExitStack  # noqa: E402

BF16 = ml_dtypes.bfloat16
F32 = np.float32
NCORE = 8
GROUP = 4  # dst-block slots per gather batch
GCAP = 4  # max chunks per call
SWDGE_SCRATCH = 65536  # descriptor-ring carveout (4096 descriptors)
SINGLE_PACKET = False  # per-descriptor packets pipeline HBM reads ~3.4x better
NQUEUES = 4  # rotate gather calls across SWDGE queues
LO_SPLIT = 32768  # int16 gather-index limit -> lo/hi table split
ALU = mybir.AluOpType
AF = mybir.ActivationFunctionType

TRACE = False  # set True (e.g. from test.py) to capture an NTFF profile
LAST_EXEC_NS = None
LAST_RESULTS = None


# --------------------------------------------------------------------------
# schedule
# --------------------------------------------------------------------------

def _make_schedule(LO, HI, nblk, group):
    """Chunk numbering shared by host prep and program build.

    Chunks are numbered [batch0: lo chunks slot-major, then hi chunks
    slot-major][batch1: ...]. Chunk c covers gather-index positions
    [128c, 128(c+1)) and metadata column c.
    """
    batches = []
    c = 0
    for s0 in range(0, nblk, group):
        slots = list(range(s0, min(s0 + group, nblk)))
        lo_entries = []
        lo_base, col = c, 0
        for j in slots:
            lo_entries.append((j, col, c, int(LO[j])))
            col += int(LO[j])
            c += int(LO[j])
        n_lo = col
        hi_entries = []
        hi_base, col = c, 0
        for j in slots:
            hi_entries.append((j, col, c, int(HI[j])))
            col += int(HI[j])
            c += int(HI[j])
        n_hi = col
        batches.append(dict(slots=slots, lo=lo_entries, hi=hi_entries,
                            n_lo=n_lo, n_hi=n_hi,
                            lo_base=lo_base, hi_base=hi_base))
    return batches, c


# --------------------------------------------------------------------------
# host-side preprocessing (sharding)
# --------------------------------------------------------------------------

def _prep(x, edge_index, W1, b1, W2, b2):
    x = np.asarray(x, F32)
    N, FIN = x.shape
    W1 = np.asarray(W1, F32)
    W2 = np.asarray(W2, F32)
    FMID = W1.shape[1]
    FOUT = W2.shape[1]
    assert N % NCORE == 0
    assert FIN == 128 and FOUT == 128 and FMID % 128 == 0
    pcr = N // NCORE                      # real nodes per core
    nblk = (pcr + 127) // 128             # dst blocks per core
    npc = nblk * 128                      # padded nodes per core

    src = np.asarray(edge_index[0], np.int64)
    dst = np.asarray(edge_index[1], np.int64)
    deg = np.bincount(dst, minlength=N).astype(np.float64) + 1.0
    inv = 1.0 / np.sqrt(deg)

    # self-loops as ordinary edges
    loops = np.arange(N, dtype=np.int64)
    srca = np.concatenate([src, loops])
    dsta = np.concatenate([dst, loops])
    w_e = (inv[srca] * inv[dsta]).astype(F32)

    # balanced node -> (block, d) assignment within each core: snake-deal
    # nodes in decreasing layer-1 lo-in-edge count so every block's lo and
    # total edge counts are nearly equal (tight chunk budgets, identity
    # slot order).
    n_lo1 = (np.bincount(dst[src < LO_SPLIT], minlength=N)
             + (np.arange(N) < LO_SPLIT))
    Bmap = np.empty(N, np.int64)
    Dmap = np.empty(N, np.int64)
    snake = np.concatenate([np.arange(nblk), np.arange(nblk)[::-1]])
    for k in range(NCORE):
        nodes = np.arange(k * pcr, (k + 1) * pcr)
        nodes = nodes[np.argsort(-n_lo1[nodes], kind="stable")]
        blk = snake[np.arange(pcr) % (2 * nblk)]
        Bmap[nodes] = blk
        dpos = np.zeros(nblk, np.int64)
        for n, b in zip(nodes, blk):
            Dmap[n] = dpos[b]
            dpos[b] += 1
        assert dpos.max() <= 128

    k_arr = dsta // pcr
    b_arr = Bmap[dsta]
    d_arr = Dmap[dsta].astype(F32)        # dst offset within block

    key = (k_arr * nblk + b_arr).astype(np.int64)
    cnt = np.bincount(key, minlength=NCORE * nblk).reshape(NCORE, nblk)

    # identity slot order: slot j == block j on every core
    perm = np.tile(np.arange(nblk), (NCORE, 1))

    # gather-table row position of each edge's source, per layer
    sk = srca // pcr
    pos = {
        1: srca,                                            # x table [N, FIN]
        2: (sk * 128 + Dmap[srca]) * nblk + Bmap[srca],     # t2_full table
    }
    tabrows = {1: N, 2: NCORE * npc}

    # presort edges by (core, block) once
    order = np.argsort(key, kind="stable")
    key_s = key[order]
    grp = np.arange(NCORE * nblk)
    starts = np.searchsorted(key_s, grp)
    ends = np.searchsorted(key_s, grp + 1)

    scheds, Cs, budgets = {}, {}, {}
    for L in (1, 2):
        lo_cnt = np.bincount(key[pos[L] < LO_SPLIT],
                             minlength=NCORE * nblk).reshape(NCORE, nblk)
        hi_cnt = cnt - lo_cnt
        lo_p = np.take_along_axis(lo_cnt, perm, 1)  # [core, slot]
        hi_p = np.take_along_axis(hi_cnt, perm, 1)
        LO = np.ceil(lo_p.max(0) / 128).astype(int)
        HI = np.ceil(hi_p.max(0) / 128).astype(int)
        budgets[L] = (tuple(LO), tuple(HI))
        scheds[L], Cs[L] = _make_schedule(LO, HI, nblk, GROUP)

    # fill per-core index + metadata arrays in schedule order
    per_core = []
    for k in range(NCORE):
        maps = {}
        for L in (1, 2):
            C = Cs[L]
            idxf = np.zeros(C * 128, np.int32)
            wf = np.zeros(C * 128, F32)
            df = np.zeros(C * 128, F32)
            pL = pos[L]
            for batch in scheds[L]:
                for kind, entries in (("lo", batch["lo"]), ("hi", batch["hi"])):
                    for (j, _col, gc, nch) in entries:
                        if nch == 0:
                            continue
                        b = perm[k, j]
                        g = k * nblk + b
                        rows = order[starts[g]:ends[g]]
                        p = pL[rows]
                        sel = rows[p < LO_SPLIT] if kind == "lo" else rows[p >= LO_SPLIT]
                        m = len(sel)
                        assert m <= nch * 128
                        base = gc * 128
                        pp = pL[sel]
                        if kind == "hi":
                            pp = pp - LO_SPLIT
                        idxf[base:base + m] = pp
                        wf[base:base + m] = w_e[sel]
                        df[base:base + m] = d_arr[sel]
            assert idxf.max(initial=0) < min(LO_SPLIT, tabrows[L]) and idxf.min(initial=0) >= 0
            # wrap indices: position i -> (partition i%16, col i//16), 8x replicated
            idx16 = np.ascontiguousarray(
                np.tile(idxf.astype(np.int16).reshape(-1, 16).T, (NCORE, 1)))
            maps[f"idx{L}"] = idx16
            maps[f"m{L}w"] = np.ascontiguousarray(wf.reshape(C, 128).T)
            maps[f"m{L}d"] = np.ascontiguousarray(df.reshape(C, 128).T)
        per_core.append(maps)

    # shared constant inputs
    nh = FMID // 128
    shared = {
        "xt": np.ascontiguousarray(x.astype(BF16)),
        "w1t": np.ascontiguousarray(W1.astype(BF16)),
        "w2t": np.ascontiguousarray(W2.astype(BF16)),
        "b1t": np.ascontiguousarray(np.asarray(b1, F32).reshape(nh, 128).T),
        "b2t": np.ascontiguousarray(np.asarray(b2, F32).reshape(1, 128).T),
        "iota": np.ascontiguousarray(
            np.tile(np.arange(128, dtype=F32), (128, 1)).astype(BF16)),
    }

    dims = dict(N=N, FIN=FIN, FMID=FMID, FOUT=FOUT,
                pcr=pcr, nblk=nblk, npc=npc)
    return dims, scheds, Cs, budgets, shared, per_core


# --------------------------------------------------------------------------
# bass program
# --------------------------------------------------------------------------

def _build(dims, scheds, Cs, collective=True, repeat=1):
    N, FIN, FMID, FOUT = dims["N"], dims["FIN"], dims["FMID"], dims["FOUT"]
    nblk, npc = dims["nblk"], dims["npc"]
    nh = FMID // 128
    dt = mybir.dt

    nc = bacc.Bacc("TRN2", num_devices=NCORE,
                   dynamic_dma_scratch_size=SWDGE_SCRATCH,
                   num_swdge_queues=NQUEUES)
    qctr = [0]

    xt = nc.declare_dram_parameter("xt", [N, FIN], dt.bfloat16, False)
    w1 = nc.declare_dram_parameter("w1t", [128, FMID], dt.bfloat16, False)
    w2 = nc.declare_dram_parameter("w2t", [FMID, FOUT], dt.bfloat16, False)
    b1 = nc.declare_dram_parameter("b1t", [128, nh], dt.float32, False)
    b2 = nc.declare_dram_parameter("b2t", [128, 1], dt.float32, False)
    iota = nc.declare_dram_parameter("iota", [128, 128], dt.bfloat16, False)
    idx_d, mw_d, md_d = {}, {}, {}
    for L in (1, 2):
        C = Cs[L]
        idx_d[L] = nc.declare_dram_parameter(f"idx{L}", [128, C * 8], dt.int16, False)
        mw_d[L] = nc.declare_dram_parameter(f"m{L}w", [128, C], dt.float32, False)
        md_d[L] = nc.declare_dram_parameter(f"m{L}d", [128, C], dt.float32, False)
    pooled_d = nc.declare_dram_parameter("pooled", [128, 1], dt.float32, True)

    t2_local = nc.dram_tensor("t2loc", [128, nblk, FOUT], dt.bfloat16)
    t2_full = nc.dram_tensor("t2full", [NCORE * npc, FOUT], dt.bfloat16,
                             addr_space="Shared")

    with TileContext(nc) as tc, ExitStack() as ctx:
        constp = ctx.enter_context(tc.tile_pool(name="const", bufs=1))
        msgp = ctx.enter_context(tc.tile_pool(name="msg", bufs=5))
        sbp = ctx.enter_context(tc.tile_pool(name="sbld", bufs=4))
        workp = ctx.enter_context(tc.tile_pool(name="work", bufs=3))
        psump = ctx.enter_context(tc.tile_pool(name="ps", bufs=2, space="PSUM"))
        psagg = ctx.enter_context(tc.tile_pool(name="psagg", bufs=4, space="PSUM"))

        def load(tag, shape, dtype, src_ap):
            t = constp.tile(shape, dtype, tag=tag)
            nc.sync.dma_start(out=t[:], in_=src_ap)
            return t

        w1_sb = load("w1c", [128, FMID], dt.bfloat16, w1[:])
        w2_sb = load("w2c", [128, nh, FOUT], dt.bfloat16,
                     w2[:].rearrange("(h k) n -> k h n", h=nh))
        b1_sb = load("b1c", [128, nh], dt.float32, b1[:])
        b2_sb = load("b2c", [128, 1], dt.float32, b2[:])
        iota_sb = load("iotac", [128, 128], dt.bfloat16, iota[:])
        idx_sb, mw_sb, md_sb = {}, {}, {}
        for L in (1, 2):
            idx_sb[L] = load(f"idx{L}c", [128, Cs[L] * 8], dt.int16, idx_d[L][:])
            mw_sb[L] = load(f"mw{L}c", [128, Cs[L]], dt.float32, mw_d[L][:])
            md_sb[L] = load(f"md{L}c", [128, Cs[L]], dt.float32, md_d[L][:])
        pooled_sb = constp.tile([128, nblk], dt.float32, tag="pooledc")

        def do_layer(L, tab_handle, tabrows):
            felem = FIN if L == 1 else FOUT
            tab_lo = tab_handle[:, :]
            tab_hi = tab_handle[LO_SPLIT:, :] if tabrows > LO_SPLIT else None
            for batch in scheds[L]:
                def gather(tab, n, base, tag):
                    # split into <=GCAP-chunk calls: a single call larger than
                    # the SWDGE descriptor ring wedges the DMA engines
                    mt = msgp.tile([128, n, felem], dt.bfloat16, tag=tag)
                    for a in range(0, n, GCAP):
                        b = min(a + GCAP, n)
                        nc.gpsimd.dma_gather(
                            mt[:, a:b, :], tab,
                            idx_sb[L][:, (base + a) * 8:(base + b) * 8],
                            (b - a) * 128, (b - a) * 128, felem,
                            single_packet=SINGLE_PACKET,
                            queue_num=qctr[0] % NQUEUES)
                        qctr[0] += 1
                    return mt

                m_lo = m_hi = None
                if batch["n_lo"]:
                    m_lo = gather(tab_lo, batch["n_lo"], batch["lo_base"], "mlo")
                if batch["n_hi"]:
                    m_hi = gather(tab_hi, batch["n_hi"], batch["hi_base"], "mhi")
                if L == 1:
                    t2b = workp.tile([128, len(batch["slots"]), FOUT],
                                     dt.bfloat16, tag="t2b")
                for jj, ((j, colL, gcL, nL), (_j2, colH, gcH, nH)) in enumerate(
                        zip(batch["lo"], batch["hi"])):
                    total = nL + nH
                    agg = psagg.tile([128, 128], dt.float32, tag="agg")
                    if total == 0:
                        nc.vector.memset(agg[:], 0.0)
                    ci = 0
                    for mt, col0, gc0, n in ((m_lo, colL, gcL, nL),
                                             (m_hi, colH, gcH, nH)):
                        for i in range(n):
                            s_t = sbp.tile([128, 128], dt.bfloat16, tag="S")
                            nc.vector.tensor_scalar(
                                out=s_t[:], in0=iota_sb[:],
                                scalar1=md_sb[L][:, gc0 + i:gc0 + i + 1],
                                scalar2=mw_sb[L][:, gc0 + i:gc0 + i + 1],
                                op0=ALU.is_equal, op1=ALU.mult)
                            nc.tensor.matmul(agg[:], lhsT=mt[:, col0 + i, :],
                                             rhs=s_t[:], start=(ci == 0),
                                             stop=(ci == total - 1))
                            ci += 1
                    if L == 1:
                        agg_sb = workp.tile([128, 128], dt.bfloat16, tag="aggsb")
                        nc.scalar.copy(agg_sb[:], agg[:])
                        h1p = psump.tile([128, nh, 128], dt.float32, tag="h1")
                        for h in range(nh):
                            nc.tensor.matmul(h1p[:, h, :],
                                             lhsT=w1_sb[:, 128 * h:128 * (h + 1)],
                                             rhs=agg_sb[:], start=True, stop=True)
                        h1_sb = workp.tile([128, nh, 128], dt.bfloat16, tag="h1sb")
                        for h in range(nh):
                            nc.scalar.activation(h1_sb[:, h, :], h1p[:, h, :],
                                                 AF.Relu, bias=b1_sb[:, h:h + 1],
                                                 scale=1.0)
                        t2p = psump.tile([128, 128], dt.float32, tag="t2")
                        for h in range(nh):
                            nc.tensor.matmul(t2p[:], lhsT=h1_sb[:, h, :],
                                             rhs=w2_sb[:, h, :],
                                             start=(h == 0), stop=(h == nh - 1))
                        nc.scalar.copy(t2b[:, jj, :], t2p[:])
                    else:
                        scr = workp.tile([128, 128], dt.bfloat16, tag="scr")
                        nc.scalar.activation(scr[:], agg[:], AF.Relu,
                                             bias=b2_sb[:, 0:1], scale=1.0,
                                             accum_out=pooled_sb[:, j:j + 1])
                if L == 1:
                    s0 = batch["slots"][0]
                    nc.sync.dma_start(
                        out=t2_local[:, s0:s0 + len(batch["slots"]), :],
                        in_=t2b[:])

        for _rep in range(repeat):
            do_layer(1, xt, N)
            if collective:
                nc.gpsimd.collective_compute(
                    "AllGather", ALU.bypass,
                    replica_groups=[list(range(NCORE))],
                    ins=[t2_local[:]], outs=[t2_full[:]])
            do_layer(2, t2_full, NCORE * npc)

        pout = workp.tile([128, 1], dt.float32, tag="po")
        nc.vector.tensor_reduce(pout[:], pooled_sb[:],
                                axis=mybir.AxisListType.X, op=ALU.add)
        nc.sync.dma_start(out=pooled_d[:], in_=pout[:])

    nc.compile()
    return nc


# --------------------------------------------------------------------------
# entry point
# --------------------------------------------------------------------------

_CACHE = {}


def _get_program(x, edge_index, W1, b1, W2, b2):
    dims, scheds, Cs, budgets, shared, per_core = _prep(
        x, edge_index, W1, b1, W2, b2)
    key = (dims["N"], dims["FIN"], dims["FMID"], dims["FOUT"],
           budgets[1], budgets[2])
    if key not in _CACHE:
        _CACHE[key] = _build(dims, scheds, Cs)
    return _CACHE[key], dims, shared, per_core


def kernel(x, edge_index, W1, b1, W2, b2, Wfc, bfc):
    global LAST_EXEC_NS, LAST_RESULTS
    nc, dims, shared, per_core = _get_program(x, edge_index, W1, b1, W2, b2)

    in_maps = []
    for k in range(NCORE):
        m = dict(shared)
        m.update(per_core[k])
        in_maps.append(m)

    kw = {}
    if TRACE:
        kw["trace"] = True
    res = run_bass_kernel_spmd(nc, in_maps, core_ids=list(range(NCORE)), **kw)
    LAST_RESULTS = res
    LAST_EXEC_NS = getattr(res, "exec_time_ns", None)

    partials = np.stack([np.asarray(res.results[k]["pooled"], np.float64)[:, 0]
                         for k in range(NCORE)])
    pooled_sum = partials.sum(0)

    # pad rows contribute exactly relu(b2) each (zero aggregation)
    n_pads = NCORE * (dims["npc"] - dims["pcr"])
    relu_b2 = np.maximum(np.asarray(b2, np.float64), 0.0)
    pooled = (pooled_sum - n_pads * relu_b2) / dims["N"]
    out = pooled @ np.asarray(Wfc, np.float64) + np.asarray(bfc, np.float64)
    return out.astype(F32)

